# revision 17
# baseline (speedup 1.0000x reference)
"""Trainium2 Bass kernel for nn_BatchProgramClassifier.

Reference computation (B=64, L=64, NPT=127, D=128, VOCAB=30000, LABELS=30):
  1. e = emb[tokens] @ Wc + bc                     per tree node
  2. h = bottom-up subtree sums of e (heap tree)   [B, L, NPT, D]
  3. enc = relu(max over nodes of h)               [B, L, D]
  4. masked single-head self-attention over L      [B, L, D]
  5. logits = (max over L) @ Wl + bl               [B, LABELS]

Sharding: data-parallel over batch, 8 batches per core across 8 cores.

Device pipeline (per core, all phases overlap under the gather):
  - dma_gather chunks (2048 rows; the last batch splits to 1024 to shrink
    the final DMA-drain backlog) pull fp16 embedding rows from HBM in
    transpose mode straight into D-major layout, 4 SWDGE queues deep (the
    descriptor-generation rate of the 4 Q7 core pairs is the kernel's wall:
    ~2.3ns/row aggregate, ~150us for the 65536 rows).
  - Tree nodes are stored slot-major (column s*64+t holds slot s of tree t)
    in shifted bit-reversed level order: level l occupies slots
    [2^l, 2^(l+1)), slot 0 is the pad, and children of a parent block are
    two contiguous half-blocks. Every subtree-sum add and every max-tree
    step is then one fully contiguous 2D DVE op (3D strided APs are ~4x
    slower), and level blocks align with gather chunks.
  - The DVE work for batch b is issued one batch LATE (during batch b+1's
    gather window), so every op in the in-order DVE queue is ready the
    moment it issues: no head-of-line blocking, no end-of-stream backlog.
  - The index table is uploaded in small-first pieces ahead of the weights
    so descriptor generation starts ~1us in and never gaps on indices.
  - Wc matmuls fill 2-bank PSUM tiles; one wide 1024-column ACT copy with
    the +bc bias drains each, halving ACT's per-op overhead.
  - Attention runs in two 4-batch halves: the first issues mid-stream
    (under the gather) as soon as batches 0-3 are encoded, only the second
    half plus the tiny logits matmul trails the last gather chunk.
"""

import math

import numpy as np

B, L, NPT, D_TREE = 64, 64, 127, 7
VOCAB, D, LABELS = 30000, 128, 30
NCORES = 8
BC = B // NCORES  # batches per core
TREES = BC * L  # trees per core (512)
SLOTS = 128  # per-tree storage (127 nodes + 1 pad)
CHUNK_TREES = 16  # trees per gather chunk
NCHUNKS = TREES // CHUNK_TREES  # 32
NIDX_CHUNK = CHUNK_TREES * SLOTS  # 2048
NIDX_TOTAL = TREES * SLOTS  # 65536
CHUNKS_PER_BATCH = L // CHUNK_TREES  # 4

_CACHE = {}


def _bitrev_slots():
    """Storage slot for each heap node 0..126: level l occupies slots
    [2^l, 2^(l+1)) with bit-reversed within-level order, slot 0 is the pad.
    Children of slot 2^l + i are slots 2^(l+1) + i and 2^(l+1) + 2^l + i,
    i.e. two contiguous half-blocks, and all level blocks are power-of-two
    aligned (chunk-aligned for the gather)."""
    slots = np.zeros(NPT, np.int64)
    for h in range(NPT):
        lvl = (h + 1).bit_length() - 1
        j = h - (2**lvl - 1)
        r = 0
        for b in range(lvl):
            r = (r << 1) | ((j >> b) & 1)
        slots[h] = 2**lvl + r
    return slots


def _build_nc():
    import concourse.bacc as bacc
    import concourse.hw_specs as hw_specs
    import concourse.mybir as mybir
    import concourse.tile as tile
    from concourse.library_config import mlp

    f32 = mybir.dt.float32
    f16 = mybir.dt.float16

    # The stock spec models SWDGE descriptor generation at 0.34ns/desc
    # (calibrated on plain DMAs); transpose-mode gathers measure ~9ns/desc
    # per queue under 4-queue load. The tile scheduler orders the in-order
    # engine queues from a CoreSim timeline built on this constant, so the
    # stock value makes it interleave the gather-paced pipeline ~10x too
    # optimistically (attention lands after every Wc matmul, tree ops heap
    # up at the tail). Patch the measured value in for scheduling, restore
    # after compile.
    _swdge_orig = hw_specs.TRN2Spec.SWDGE_NS_PER_DESCRIPTOR
    hw_specs.TRN2Spec.SWDGE_NS_PER_DESCRIPTOR = 9.1
    nc = bacc.Bacc(
        "TRN2",
        target_bir_lowering=False,
        debug=False,
        num_devices=NCORES,
        num_swdge_queues=4,
    )

    emb_d = nc.dram_tensor("emb", [VOCAB, D], f16, kind="ExternalInput")
    idx_d = nc.dram_tensor(
        "idxs", [128, NIDX_TOTAL // 16], mybir.dt.int16, kind="ExternalInput"
    )
    nmaskT_d = nc.dram_tensor("nmaskT", [L, BC * L], f16, kind="ExternalInput")
    wc_d = nc.dram_tensor("wc", [D, D], f32, kind="ExternalInput")
    bcv_d = nc.dram_tensor("bcv", [D, 1], f32, kind="ExternalInput")
    wq_d = nc.dram_tensor("wq", [D, D], f32, kind="ExternalInput")
    wk_d = nc.dram_tensor("wk", [D, D], f32, kind="ExternalInput")
    wv_d = nc.dram_tensor("wv", [D, D], f32, kind="ExternalInput")
    wo_d = nc.dram_tensor("wo", [D, D], f32, kind="ExternalInput")
    wl_d = nc.dram_tensor("wl", [D, LABELS], f32, kind="ExternalInput")
    blb_d = nc.dram_tensor("blb", [BC, LABELS], f32, kind="ExternalInput")
    ident_d = nc.dram_tensor("ident", [128, 128], f32, kind="ExternalInput")
    out_d = nc.dram_tensor("out", [BC, LABELS], f32, kind="ExternalOutput")

    inv_sqrt_d = 1.0 / math.sqrt(float(D))

    with tile.TileContext(nc) as tc:
        with (
            tc.tile_pool(name="const", bufs=1) as cpool,
            tc.tile_pool(name="epool", bufs=10) as epool,
            tc.tile_pool(name="eblk", bufs=4) as eblkpool,
            tc.tile_pool(name="tpsum", bufs=3, space="PSUM") as tpsum,
            tc.tile_pool(name="apool", bufs=3) as apool,
            tc.tile_pool(name="apsum", bufs=1, space="PSUM") as apsum,
        ):
            nc.gpsimd.load_library(mlp)

            def load_const(dram, shape, dtype):
                t = cpool.tile(shape, dtype, tag=dram.name)
                if dtype == dram.dtype:
                    nc.sync.dma_start(out=t[:], in_=dram[:])
                else:
                    raw = cpool.tile(shape, dram.dtype, tag=dram.name + "_raw")
                    nc.sync.dma_start(out=raw[:], in_=dram[:])
                    nc.scalar.copy(out=t[:], in_=raw[:])
                return t

            # Index pieces first (small-first, so chunk-0 descriptor
            # generation starts ~1us in and later chunks never wait on
            # idxs), then Wc/bc (first matmul fires ~15us in), then the
            # attention weights (needed only ~80us in).
            idx_t = cpool.tile([128, NIDX_TOTAL // 16], mybir.dt.int16, tag="idxs")
            idx_cols = NIDX_CHUNK // 16  # 128 cols per gather chunk

            def idx_pieces(pieces):  # sizes in gather-chunk units
                nonlocal_pos = idx_pieces.pos
                for np_ in pieces:
                    nc.sync.dma_start(
                        out=idx_t[
                            :, nonlocal_pos * idx_cols : (nonlocal_pos + np_) * idx_cols
                        ],
                        in_=idx_d[
                            :, nonlocal_pos * idx_cols : (nonlocal_pos + np_) * idx_cols
                        ],
                    )
                    nonlocal_pos += np_
                idx_pieces.pos = nonlocal_pos

            idx_pieces.pos = 0
            idx_pieces([1, 7])  # chunks for batches 0-1 land in ~6us
            wc_t = load_const(wc_d, [D, D], f16)
            bcv_t = load_const(bcv_d, [D, 1], f32)
            idx_pieces([4] * ((NCHUNKS - 8) // 4))  # the rest
            wq_t = load_const(wq_d, [D, D], f16)
            wk_t = load_const(wk_d, [D, D], f16)
            wv_t = load_const(wv_d, [D, D], f16)
            wo_t = load_const(wo_d, [D, D], f16)
            wl_t = load_const(wl_d, [D, LABELS], f16)
            blb_t = load_const(blb_d, [BC, LABELS], f32)
            ident_t = load_const(ident_d, [128, 128], f32)
            nmaskT_t = load_const(nmaskT_d, [L, BC * L], f16)
            ident16 = cpool.tile([128, 128], f16, tag="ident16")
            nc.scalar.copy(out=ident16[:], in_=ident_t[:])

            pooled_all = cpool.tile([D, BC], f16, tag="pooled")
            enc_all = cpool.tile([D, BC * L], f16, tag="encall")

            def attn_half(b0, nb):
                """Masked self-attention for batches [b0, b0+nb) into
                pooled_all[:, b0:b0+nb]. Mask folded in via constant matmuls
                of the transposed 0/-3e4 mask, exp straight off PSUM, v^T
                computed directly as enc^T @ Wv."""
                W = nb * L
                e0 = b0 * L
                enc_h = enc_all[:, e0 : e0 + W]
                qp = apsum.tile([D, 512], f32, tag="bankA")
                nc.tensor.matmul(
                    qp[:, :W], lhsT=wq_t[:], rhs=enc_h, start=True, stop=True
                )
                qs = apool.tile([D, 512], f16, tag="qs")
                nc.scalar.mul(qs[:, :W], qp[:, :W], inv_sqrt_d)
                kp = apsum.tile([D, 512], f32, tag="bankB")
                nc.tensor.matmul(
                    kp[:, :W], lhsT=wk_t[:], rhs=enc_h, start=True, stop=True
                )
                ks = apool.tile([D, 512], f16, tag="ks")
                nc.scalar.copy(out=ks[:, :W], in_=kp[:, :W])

                scp = apsum.tile([D, 512], f32, tag="bankA")
                for i in range(nb):
                    b = b0 + i
                    s = scp[:L, i * L : (i + 1) * L]
                    nc.tensor.matmul(
                        s,
                        lhsT=qs[:, i * L : (i + 1) * L],
                        rhs=ks[:, i * L : (i + 1) * L],
                        start=True,
                        stop=False,
                    )
                    nc.tensor.matmul(
                        s,
                        lhsT=nmaskT_t[:, b * L : (b + 1) * L],
                        rhs=ident16[:L, :L],
                        start=False,
                        stop=True,
                    )
                ex = apool.tile([L, 512], f32, tag="ex")
                nc.scalar.activation(
                    ex[:, :W], scp[:L, :W], mybir.ActivationFunctionType.Exp
                )
                rsum = apool.tile([L, BC], f32, tag="rsum")
                nc.vector.reduce_sum(
                    out=rsum[:, :nb],
                    in_=ex[:, :W].rearrange("q (b k) -> q b k", k=L),
                    axis=mybir.AxisListType.X,
                )
                rinv = apool.tile([L, BC], f32, tag="rinv")
                nc.vector.reciprocal(rinv[:, :nb], rsum[:, :nb])
                attn = apool.tile([L, 512], f32, tag="attn")
                nc.vector.tensor_mul(
                    out=attn[:, :W].rearrange("q (b k) -> q b k", k=L),
                    in0=ex[:, :W].rearrange("q (b k) -> q b k", k=L),
                    in1=rinv[:, :nb, None].to_broadcast((L, nb, L)),
                )

                # v^T directly per batch: vtp[l, d'] = sum_d enc[d, l] Wv[d, d']
                vts = apool.tile([L, BC * D], f16, tag="vts")
                for r0 in range(0, nb, 4):
                    rn = min(4, nb - r0)
                    vtp = apsum.tile([D, 512], f32, tag="bankB")
                    for i in range(rn):
                        b = b0 + r0 + i
                        nc.tensor.matmul(
                            vtp[:L, i * D : (i + 1) * D],
                            lhsT=enc_all[:, b * L : (b + 1) * L],
                            rhs=wv_t[:],
                            start=True,
                            stop=True,
                        )
                    nc.scalar.copy(
                        out=vts[:, r0 * D : (r0 + rn) * D], in_=vtp[:L, : rn * D]
                    )
                atp = apsum.tile([D, 512], f32, tag="bankA")
                for i in range(nb):
                    nc.tensor.transpose(
                        atp[:L, i * L : (i + 1) * L],
                        attn[:, i * L : (i + 1) * L],
                        ident_t[:L, :L],
                    )
                ats = apool.tile([L, 512], f16, tag="ats")
                nc.scalar.copy(out=ats[:, :W], in_=atp[:L, :W])

                op = apsum.tile([D, 512], f32, tag="bankB")
                for i in range(nb):
                    nc.tensor.matmul(
                        op[:, i * L : (i + 1) * L],
                        lhsT=vts[:, i * D : (i + 1) * D],
                        rhs=ats[:, i * L : (i + 1) * L],
                        start=True,
                        stop=True,
                    )
                os_ = apool.tile([D, 512], f16, tag="os")
                nc.scalar.copy(out=os_[:, :W], in_=op[:, :W])
                o2p = apsum.tile([D, 512], f32, tag="bankA")
                nc.tensor.matmul(
                    o2p[:, :W], lhsT=wo_t[:], rhs=os_[:, :W], start=True, stop=True
                )
                nc.vector.reduce_max(
                    out=pooled_all[:, b0 : b0 + nb],
                    in_=o2p[:, :W].rearrange("d (b l) -> d b l", l=L),
                    axis=mybir.AxisListType.X,
                )

            def tree_pre(eb):
                """Level-5 adds + leaf max-fold into the pad slot: needs only
                chunks 1-3 (slots 32..128)."""
                p0, c0, w = 32 * L, 64 * L, 32 * L
                nc.vector.tensor_add(
                    out=eb[:, p0 : p0 + w],
                    in0=eb[:, p0 : p0 + w],
                    in1=eb[:, c0 : c0 + w],
                )
                nc.vector.tensor_add(
                    out=eb[:, p0 : p0 + w],
                    in0=eb[:, p0 : p0 + w],
                    in1=eb[:, c0 + w : c0 + 2 * w],
                )
                s = 32
                while s >= 2:
                    nc.vector.tensor_max(
                        out=eb[:, 64 * L : (64 + s) * L],
                        in0=eb[:, 64 * L : (64 + s) * L],
                        in1=eb[:, (64 + s) * L : (64 + 2 * s) * L],
                    )
                    s //= 2
                nc.vector.tensor_max(
                    out=eb[:, 0:L],
                    in0=eb[:, 64 * L : 65 * L],
                    in1=eb[:, 65 * L : 66 * L],
                )

            def tree_post(b, eb):
                """Levels 4..0 adds (need chunk 0: slots 1..32), slot fold,
                ReLU into enc_all."""
                for lvl in range(D_TREE - 3, -1, -1):
                    p0 = (2**lvl) * L
                    c0 = (2 ** (lvl + 1)) * L
                    w = (2**lvl) * L
                    nc.vector.tensor_add(
                        out=eb[:, p0 : p0 + w],
                        in0=eb[:, p0 : p0 + w],
                        in1=eb[:, c0 : c0 + w],
                    )
                    nc.vector.tensor_add(
                        out=eb[:, p0 : p0 + w],
                        in0=eb[:, p0 : p0 + w],
                        in1=eb[:, c0 + w : c0 + 2 * w],
                    )
                s = 32
                while s >= 1:
                    nc.vector.tensor_max(
                        out=eb[:, 0 : s * L],
                        in0=eb[:, 0 : s * L],
                        in1=eb[:, s * L : 2 * s * L],
                    )
                    s //= 2
                nc.vector.tensor_scalar_max(
                    enc_all[:, b * L : (b + 1) * L], eb[:, 0:L], 0.0
                )

            prev = None
            for b in range(BC):
                # one eb super-tile per batch: 64 trees x 128 slots, f16
                eb = eblkpool.tile([128, L * SLOTS], f16, tag="eb")
                # last batch splits its gathers in half so the tail's
                # DMA-drain backlog (packets trail descriptor gen) is
                # smaller; every extra gather instruction costs ~1us of
                # descriptor-generation, so nothing else is split
                nsub = 2 if b == BC - 1 else 1
                sub_idx = NIDX_CHUNK // nsub
                # last batch gathers chunk 0 (slots 0..32, needed only by
                # tree_post) LAST, so tree_pre(b7) overlaps its gather
                order = (1, 2, 3, 0) if b == BC - 1 else (0, 1, 2, 3)
                for k in order:
                    c = b * CHUNKS_PER_BATCH + k
                    for s_ in range(nsub):
                        et = epool.tile([128, 1, sub_idx], f16, tag="et")
                        i0 = c * idx_cols + s_ * (sub_idx // 16)
                        nc.gpsimd.dma_gather(
                            et[:],
                            emb_d[:],
                            idx_t[:, i0 : i0 + sub_idx // 16],
                            sub_idx,
                            sub_idx,
                            D,
                            transpose=True,
                            single_packet=False,
                            queue_num=(c * nsub + s_) % 4,
                        )
                        for j in range(sub_idx // 1024):
                            # two bank-sized matmuls fill a 2-bank PSUM
                            # tile; ONE wide ACT copy (with the +bc bias)
                            # drains it, halving ACT's per-op overhead
                            pp = tpsum.tile([128, 1024], f32, tag="pp")
                            for h in range(2):
                                nc.tensor.matmul(
                                    pp[:, h * 512 : (h + 1) * 512],
                                    lhsT=wc_t[:],
                                    rhs=et[
                                        :,
                                        0,
                                        j * 1024 + h * 512 : j * 1024 + (h + 1) * 512,
                                    ],
                                    start=True,
                                    stop=True,
                                )
                            off = k * NIDX_CHUNK + s_ * sub_idx + j * 1024
                            # skip the pad-slot columns so tree_pre's leaf-max
                            # write there never serializes against this drain
                            skip = L if off == 0 else 0
                            nc.scalar.activation(
                                eb[:, off + skip : off + 1024],
                                pp[:, skip:1024],
                                mybir.ActivationFunctionType.Identity,
                                bias=bcv_t[:],
                                scale=1.0,
                            )

                # one-batch-lag DVE: the previous batch is fully drained by
                # now, so the in-order DVE queue never head-of-line blocks
                if prev is not None:
                    tree_pre(prev[1])
                    tree_post(prev[0], prev[1])
                    # attention trails encoding in pieces so only batch 7's
                    # chain (issued after the loop) sits past the last gather
                    if prev[0] == 3:
                        attn_half(0, 4)
                    elif prev[0] == 5:
                        attn_half(4, 2)
                    elif prev[0] == 6:
                        attn_half(6, 1)
                prev = (b, eb)
            # tail: tree_pre(b7) overlaps the chunk-0 gather, then the short
            # chunk-0-dependent half and batch 7's attention finish it off
            tree_pre(prev[1])
            tree_post(prev[0], prev[1])
            attn_half(BC - 1, 1)

            # ---- logits ----
            lbank = apsum.tile([D, 512], f32, tag="bankB")
            lgp = lbank[:BC, :LABELS]
            nc.tensor.matmul(
                lgp[:], lhsT=pooled_all[:], rhs=wl_t[:], start=True, stop=True
            )
            outs = apool.tile([BC, LABELS], f32, tag="outs")
            nc.vector.tensor_add(out=outs[:], in0=lgp[:], in1=blb_t[:])
            nc.sync.dma_start(out=out_d[:], in_=outs[:])

    try:
        nc.compile()
    finally:
        hw_specs.TRN2Spec.SWDGE_NS_PER_DESCRIPTOR = _swdge_orig
    return nc


def _get_nc():
    if "nc" not in _CACHE:
        _CACHE["nc"] = _build_nc()
    return _CACHE["nc"]


def kernel(tokens, mask, emb, Wc, bc, Wq, Wk, Wv, Wo, Wl, bl, _trace=False):
    from concourse.bass_utils import run_bass_kernel_spmd

    tokens = np.asarray(tokens)
    mask = np.asarray(mask)
    emb16 = np.asarray(emb, dtype=np.float32).astype(np.float16)

    blb = np.tile(np.asarray(bl, np.float32)[None, :], (BC, 1))

    common = {
        "emb": emb16,
        "wc": np.asarray(Wc, np.float32),
        "bcv": np.asarray(bc, np.float32).reshape(D, 1),
        "wq": np.asarray(Wq, np.float32),
        "wk": np.asarray(Wk, np.float32),
        "wv": np.asarray(Wv, np.float32),
        "wo": np.asarray(Wo, np.float32),
        "wl": np.asarray(Wl, np.float32),
        "blb": blb,
        "ident": np.eye(128, dtype=np.float32),
    }

    slots = _bitrev_slots()  # heap node -> storage slot

    in_maps = []
    for c in range(NCORES):
        tok_c = np.asarray(tokens[c * BC : (c + 1) * BC]).reshape(TREES, NPT)
        slotted = np.zeros((TREES, SLOTS), tok_c.dtype)
        slotted[:, slots] = tok_c  # pad slot 0 keeps idx 0
        # slot-major within each batch: position (b, s, t) = b*8192 + s*64 + t
        idx_lin = (
            slotted.reshape(BC, L, SLOTS).transpose(0, 2, 1).reshape(-1)
        )
        idx_arr = np.tile(
            idx_lin.astype(np.int16).reshape(-1, 16).T, (8, 1)
        )  # [128, NIDX_TOTAL/16]
        # nmaskT[k, b*L + q] = 0 where mask[b, q, k] > 0 else -3e4
        mask_c = np.asarray(mask[c * BC : (c + 1) * BC]) > 0
        nmaskT = ((mask_c.astype(np.float32) - 1.0) * 3e4).transpose(2, 0, 1)
        nmaskT = np.ascontiguousarray(nmaskT.reshape(L, BC * L)).astype(np.float16)
        in_maps.append({**common, "idxs": idx_arr, "nmaskT": nmaskT})

    nc = _get_nc()
    res = run_bass_kernel_spmd(
        nc, in_maps, core_ids=list(range(NCORES)), trace=_trace
    )
    out = np.concatenate([r["out"] for r in res.results], axis=0)  # [B, LABELS]
    if _trace:
        return out, res
    return out
